# revision 28
# baseline (speedup 1.0000x reference)
"""Trainium2 Bass kernel for the KGTM-style GRU message-passing GNN.

Reference math (per time step, T=3):
    agg_in  = A_in  @ nodes          (per batch)
    agg_out = A_in.T @ nodes
    zv = sigmoid(agg_in@W3wa.T + agg_out@W3wb.T + fn@W3u.T)
    rv = sigmoid(agg_in@W4wa.T + agg_out@W4wb.T + fn@W4u.T)
    hv = tanh   (agg_in@W5wa.T + agg_out@W5wb.T + (rv*fn)@W5u.T)
    fn' = (1-zv)*fn + zv*hv = hv - q*(hv - fn)   with q = 1-zv
    out_t = fn'@Wouta.T + x@Woutb.T + b_out

Mapping: pure data parallel over batch (8 cores x 256 batches, padded to 258
= 43 tiles of 6).  On-chip layout "L2" puts (batch-local, channel) on the
128-partition axis (6*20 = 120 partitions) and the node index n (512) on the
free axis; layout "L1" is the transpose ([node m, (b,h)]), used as the
stationary operand of the aggregation so agg lands directly in L2.

Precision/engine scheme (cost-model driven):
  - Aggregation + z/r/h matmuls run as fp8e4 DoubleRow (2 k-tiles per mm,
    0.5 cycles/row).  Scales: A*256, agg-cast*(16/256), gate weights *32,
    fn-side weights *512; the sigmoid/tanh activation descales by 1/512.
    r's stationaries are negated so one fused sigmoid over (q|r) with
    scale=-1/512 yields q=1-z and r.
  - The output projection runs in bf16 (direct output path needs accuracy).
  - DVE does the bf16 state chain (hmf=h-fn, m=q*hmf, fn'=h-m) at 2x rate
    plus the fp8 recast of fn'; Pool (gpsimd) does the agg fp8 cast; Act
    does sigmoid/tanh/transpose-evac/ox-evac.
  - fn' returns to L1 for the next aggregation via 4 fp8 PE transposes.
"""

import os
import numpy as np
import ml_dtypes

import concourse.bacc as bacc
import concourse.tile as tile
import concourse.mybir as mybir
from concourse.bass_utils import run_bass_kernel_spmd

F32 = mybir.dt.float32
BF16 = mybir.dt.bfloat16
F16 = mybir.dt.float16
FP8 = mybir.dt.float8e4
AF = mybir.ActivationFunctionType
ALU = mybir.AluOpType
DR = mybir.MatmulPerfMode.DoubleRow

E4NP = ml_dtypes.float8_e4m3
BFNP = ml_dtypes.bfloat16

B, N, H, T = 2048, 512, 20, 3
NCORES = 8
BS = B // NCORES          # 256 batches per core
BPER = 6                  # batches per partition tile
TP = BPER * H             # 120 partitions per tile
NT = 43                   # tiles per core (43*6 = 258, 2 batches of zero pad)
BPAD = NT * BPER          # 258
MK = N // 128             # 4 m-chunks of 128
WSTRIDE = 4               # waves between consecutive steps of one tile

SA = 256.0                # A scale
SAGG = 16.0               # agg fp8 scale
SW = 32.0                 # gate agg-side weight scale
SG = SAGG * SW            # gate psum scale (fn-side weights use this)

LAST_RESULTS = None


ABL = set(os.environ.get("KABL", "").split(","))


def build_nc():
    nc = bacc.Bacc("TRN2", target_bir_lowering=False, debug=False,
                   num_devices=NCORES)

    xl1_d = nc.dram_tensor("xl1", [NT, 128, MK, 128], FP8, kind="ExternalInput")
    xbf_d = nc.dram_tensor("xbf", [NT, TP + 1, N], BF16, kind="ExternalInput")
    atk_d = nc.dram_tensor("atk", [128, MK, N], FP8, kind="ExternalInput")
    ak_d = nc.dram_tensor("ak", [128, MK, N], FP8, kind="ExternalInput")
    # fp8 DoubleRow gate stationaries [K=120, 2, M=120]
    w8names = ["wz_ag", "wr_ag", "wh_ag"]
    w8_d = {w: nc.dram_tensor(w, [TP, 2, 128], FP8, kind="ExternalInput")
            for w in w8names}
    woa_d = nc.dram_tensor("woa", [TP, TP], BF16, kind="ExternalInput")
    wob_d = nc.dram_tensor("wob", [TP + 1, TP], BF16, kind="ExternalInput")
    whu_d = nc.dram_tensor("whu", [TP, TP], BF16, kind="ExternalInput")
    wzu_d = nc.dram_tensor("wzu", [TP, TP], BF16, kind="ExternalInput")
    wru_d = nc.dram_tensor("wru", [TP, TP], BF16, kind="ExternalInput")
    ident_d = nc.dram_tensor("ident", [128, 128], BF16, kind="ExternalInput")
    out_d = nc.dram_tensor("out", [NT, TP, T, N], F16, kind="ExternalOutput")

    with tile.TileContext(nc) as tc:
        with (
            tc.tile_pool(name="const", bufs=1) as cpool,
            tc.tile_pool(name="state", bufs=2 * WSTRIDE + 4) as spool,
            tc.tile_pool(name="work", bufs=4) as wpool,
            tc.tile_pool(name="psA", bufs=1, space="PSUM") as psA,
            tc.tile_pool(name="psB", bufs=1, space="PSUM") as psB,
            tc.tile_pool(name="psO", bufs=2, space="PSUM") as psO,
        ):
            # ---- constants ----
            atk = cpool.tile([128, MK, N], FP8, name="atk")
            ak = cpool.tile([128, MK, N], FP8, name="ak")
            nc.sync.dma_start(atk[:], atk_d.ap())
            nc.sync.dma_start(ak[:], ak_d.ap())
            w8 = {}
            for w in w8names:
                w8[w] = cpool.tile([TP, 2, 128], FP8, name=f"{w}_sb")
                nc.sync.dma_start(w8[w][:], w8_d[w].ap())
            woa = cpool.tile([TP, TP], BF16, name="woa")
            wob = cpool.tile([TP + 1, TP], BF16, name="wob")
            whu = cpool.tile([TP, TP], BF16, name="whu")
            wzu = cpool.tile([TP, TP], BF16, name="wzu")
            wru = cpool.tile([TP, TP], BF16, name="wru")
            nc.sync.dma_start(woa[:], woa_d.ap())
            nc.sync.dma_start(wob[:], wob_d.ap())
            nc.sync.dma_start(whu[:], whu_d.ap())
            nc.sync.dma_start(wzu[:], wzu_d.ap())
            nc.sync.dma_start(wru[:], wru_d.ap())
            ident = cpool.tile([128, 128], BF16, name="ident")
            nc.sync.dma_start(ident[:], ident_d.ap())

            st = [dict() for _ in range(NT)]

            def emit_loads(i):
                xl1 = spool.tile([128, MK, 128], FP8, name="xl1_sb")
                nc.sync.dma_start(xl1[:], xl1_d.ap()[i])
                fnb = spool.tile([TP + 1, N], BF16, name="fnb_sb")
                nc.sync.dma_start(fnb[:], xbf_d.ap()[i])
                xb2 = spool.tile([TP + 1, N], BF16, name="xb2_sb")
                nc.sync.dma_start(xb2[:], xbf_d.ap()[i])
                osb = spool.tile([TP, T, N], F16, name="osb_sb")
                st[i].update(xl1=xl1, fnb=fnb, osb=osb, xb2=xb2)

            def emit_store(i):
                nc.sync.dma_start(out_d.ap()[i], st[i]["osb"][:])

            def emit_step(i, t):
                xl1 = st[i]["xl1"]
                fnb = st[i]["fnb"]
                osb = st[i]["osb"]
                xb2 = st[i]["xb2"]
                fnl1 = st[i].get("fnl1")
                lhs = xl1 if t == 0 else fnl1

                # aggregation (DoubleRow over m); separate psum tags so the
                # in/out chains pipeline independently; casts split DVE/Pool
                agi_ps = psA.tile([128, N], F32, name="agi_ps", tag="agi")
                ago_ps = psA.tile([128, N], F32, name="ago_ps", tag="ago")
                for k2 in range(2):
                    nc.tensor.matmul(agi_ps[:], lhs[:, 2*k2:2*k2+2, :],
                                     atk[:, 2*k2:2*k2+2, :], perf_mode=DR,
                                     start=(k2 == 0), stop=(k2 == 1))
                for k2 in range(2):
                    nc.tensor.matmul(ago_ps[:], lhs[:, 2*k2:2*k2+2, :],
                                     ak[:, 2*k2:2*k2+2, :], perf_mode=DR,
                                     start=(k2 == 0), stop=(k2 == 1))
                agg = wpool.tile([TP, 2, N], FP8, name="agg_sb")
                nc.vector.tensor_scalar_mul(agg[:, 0, :], agi_ps[0:TP, :], SAGG / SA)
                nc.vector.tensor_scalar_mul(agg[:, 1, :], ago_ps[0:TP, :], SAGG / SA)

                # gates: separate z/r psum tags; q = sigmoid(-zpre) via the
                # negative activation scale; fn-parts in bf16
                r_ps = psB.tile([128, N], F32, name="r_ps", tag="r")
                nc.tensor.matmul(r_ps[:], w8["wr_ag"][:], agg[:],
                                 perf_mode=DR, start=True, stop=False)
                nc.tensor.matmul(r_ps[0:TP, :], wru[:], fnb[0:TP, :],
                                 start=False, stop=True)
                z_ps = psB.tile([128, N], F32, name="z_ps", tag="z")
                nc.tensor.matmul(z_ps[:], w8["wz_ag"][:], agg[:],
                                 perf_mode=DR, start=True, stop=False)
                nc.tensor.matmul(z_ps[0:TP, :], wzu[:], fnb[0:TP, :],
                                 start=False, stop=True)
                qr = wpool.tile([TP, 2, N], BF16, name="qr_sb")
                nc.scalar.activation(qr[:, 1, :], r_ps[0:TP, :], AF.Sigmoid, scale=-1.0 / SG)
                nc.scalar.activation(qr[:, 0, :], z_ps[0:TP, :], AF.Sigmoid, scale=-1.0 / SG)

                # h: fp8 agg part + bf16 (r*fn) part, tanh scale 1/SG
                rf = wpool.tile([TP, N], BF16, name="rf_sb")
                nc.gpsimd.tensor_mul(rf[:], qr[:, 1, :], fnb[0:TP, :])
                h_ps = psB.tile([128, N], F32, name="h_ps", tag="h")
                nc.tensor.matmul(h_ps[:], w8["wh_ag"][:], agg[:],
                                 perf_mode=DR, start=True, stop=False)
                nc.tensor.matmul(h_ps[0:TP, :], whu[:], rf[:],
                                 start=False, stop=True)
                h = wpool.tile([TP, N], BF16, name="h_sb")
                nc.scalar.activation(h[:], h_ps[0:TP, :], AF.Tanh, scale=1.0 / SG)

                # state update (bf16, DVE 2x): fn' = h - q*(h - fn)
                if "nochain" in ABL:
                    nc.vector.tensor_copy(fnb[0:TP, :], h[:])
                else:
                    hmf = wpool.tile([TP, N], BF16, name="hmf_sb")
                    nc.vector.tensor_sub(hmf[:], h[:], fnb[0:TP, :])
                    mq = wpool.tile([TP, N], BF16, name="mq_sb")
                    nc.vector.tensor_mul(mq[:], qr[:, 0, :], hmf[:])
                    nc.vector.tensor_sub(fnb[0:TP, :], h[:], mq[:])

                # output: o = Woa@fn' (+ ox) -> f16
                if "noo" not in ABL:
                    o_ps = psO.tile([TP, N], F32, name="o_ps", tag="o")
                    nc.tensor.matmul(o_ps[:], woa[:], fnb[0:TP, :],
                                     start=True, stop=False)
                    nc.tensor.matmul(o_ps[:], wob[:], xb2[:],
                                     start=False, stop=True)
                    nc.scalar.copy(osb[:, t, :], o_ps[:])

                # fn' -> L1 for next aggregation (fp8 PE transposes)
                if t < T - 1:
                    if "notp" in ABL:
                        st[i]["fnl1"] = xl1
                    else:
                        tp_ps = psA.tile([128, MK, TP], BF16, name="tp_ps", tag="tp")
                        fnl1 = spool.tile([128, MK, 128], FP8, name="fnl1_sb")
                        for k in range(MK):
                            nc.tensor.transpose(
                                tp_ps[:, k, :],
                                fnb[0:TP, 128*k:128*(k+1)],
                                ident[0:TP, 0:TP])
                        nc.vector.memset(fnl1[:, :, TP:128], 0)
                        nc.vector.tensor_copy(fnl1[:, :, 0:TP], tp_ps[:])
                        st[i]["fnl1"] = fnl1

            PF = 4      # waves of input prefetch
            DEF = 2     # waves the output store is deferred
            for w in range(NT + 2 * WSTRIDE + DEF + 1 + PF):
                i = w - PF
                if 0 <= i < NT:
                    emit_loads(i)
                j = w - 2 * WSTRIDE - DEF - PF
                if 0 <= j < NT:
                    emit_store(j)
                for t in range(T):
                    i = w - t * WSTRIDE - PF
                    if 0 <= i < NT:
                        emit_step(i, t)

    nc.compile()
    return nc


_NC_CACHE = None


def _get_nc():
    global _NC_CACHE
    if _NC_CACHE is None:
        _NC_CACHE = build_nc()
    return _NC_CACHE


def _q8(x, scale=1.0):
    return np.clip(np.asarray(x, np.float32) * scale, -240, 240).astype(E4NP)


def _kron6(w):
    return np.kron(np.eye(BPER, dtype=np.float32), np.asarray(w, np.float32).T)


def _host_prep(x, A_in, W3w, W3u, W4w, W4u, W5w, W5u, W_out, b_out):
    f32 = np.float32
    A_in = np.asarray(A_in, f32)

    def achunks(a):  # [N,N] -> [128, MK, N], m = 128*k + p, fp8 scaled
        return np.ascontiguousarray(
            _q8(a, SA).reshape(MK, 128, N).transpose(1, 0, 2))

    def pair8(wa, wb, s):  # [K,2,128] fp8 DoubleRow stationary (M padded)
        p = np.stack([_kron6(wa), _kron6(wb)], axis=1)   # [120, 2, 120]
        p = np.pad(p, ((0, 0), (0, 0), (0, 128 - TP)))
        return np.ascontiguousarray(_q8(p, s))

    zeros = np.zeros((H, H), f32)
    shared = {
        "atk": achunks(A_in.T),
        "ak": achunks(A_in),
        "wz_ag": pair8(W3w[:, :H], W3w[:, H:], SW),
        "wr_ag": pair8(-W4w[:, :H], -W4w[:, H:], SW),
        "wh_ag": pair8(W5w[:, :H], W5w[:, H:], SW),
        "woa": np.ascontiguousarray(_kron6(W_out[:, :H]).astype(BFNP)),
        "wob": np.ascontiguousarray(np.concatenate(
            [_kron6(W_out[:, H:]),
             np.tile(np.asarray(b_out, f32), BPER)[None, :]], axis=0
        ).astype(BFNP)),
        "whu": np.ascontiguousarray((_kron6(W5u) * SG).astype(BFNP)),
        "wzu": np.ascontiguousarray((_kron6(W3u) * SG).astype(BFNP)),
        "wru": np.ascontiguousarray((_kron6(-W4u) * SG).astype(BFNP)),
        "ident": np.eye(128, dtype=f32).astype(BFNP),
    }

    in_maps = []
    x = np.asarray(x, f32)
    for c in range(NCORES):
        xp = np.zeros((BPAD, N, H), f32)
        xp[:BS] = x[BS * c:BS * (c + 1)]
        # L1: [m, (b,h)] -> [NT, 128(p), MK(k), TP], m = 128k+p
        l1 = xp.transpose(1, 0, 2).reshape(N, NT, TP).transpose(1, 0, 2)
        l1 = l1.reshape(NT, MK, 128, TP).transpose(0, 2, 1, 3)
        l1 = np.pad(l1, ((0, 0), (0, 0), (0, 0), (0, 128 - TP)))
        # L2: [(b,h), n] -> [NT, TP, N]
        l2 = xp.transpose(0, 2, 1).reshape(NT, TP, N)
        l2e = np.concatenate(
            [l2, np.ones((NT, 1, N), f32)], axis=1)     # ones row for bias
        in_maps.append({
            "xl1": np.ascontiguousarray(_q8(l1)),
            "xbf": np.ascontiguousarray(l2e.astype(BFNP)),
            **shared})
    return in_maps


def kernel(x, A_in, W3w, W3u, W4w, W4u, W5w, W5u, W_out, b_out):
    global LAST_RESULTS
    nc = _get_nc()
    in_maps = _host_prep(x, A_in, W3w, W3u, W4w, W4u, W5w, W5u, W_out, b_out)
    res = run_bass_kernel_spmd(nc, in_maps, core_ids=list(range(NCORES)))
    LAST_RESULTS = res
    outs = []
    for c in range(NCORES):
        o = np.asarray(res.results[c]["out"], np.float32)  # [NT, TP, T, N]
        o = o.reshape(NT, BPER, H, T, N).transpose(3, 0, 1, 4, 2)
        outs.append(o.reshape(T, BPAD, N, H)[:, :BS])
    return np.ascontiguousarray(np.concatenate(outs, axis=1))
